# revision 9
# baseline (speedup 1.0000x reference)
"""Contrastive (MixAware) loss kernel for Trainium2, 8 NeuronCores.

Strategy (row-parallel with all-gathered normalized embeddings):
  - x = representations [2B, D] with B=4096, D=256. Rows are split:
    queries q = x[:B], positives p = x[B:].
  - Shard query rows across 8 cores (512 rows each). Core c also gets its
    positive rows p_c.
  - Each core: normalizes its OWN 512 query rows (which are everyone's key
    columns), casts to bf16, transposes via the PE, and AllGathers the
    [D, 512] normalized transposed shard -> full K^T [D, B] in bf16.
  - Each core computes its [512, B] block of the cosine-similarity matrix
    as bf16 matmuls into PSUM, applies exp(./T_NEG) on the Scalar engine
    with fused row-sum accumulation (accum_out), subtracts the diagonal
    term exp(sim_ii/T_NEG) (computed locally from the same bf16 values),
    and computes the positive dot products in fp32.
  - Host sums: loss = sum(log(denom) - pos/T_POS) / B.
"""

import numpy as np

import concourse.bass as bass
import concourse.mybir as mybir
import concourse.tile as tile
from concourse import bacc
from concourse.bass_utils import run_bass_kernel_spmd
from concourse.masks import make_identity

B = 4096
D = 256
NCORES = 8
RPC = B // NCORES        # 512 rows (queries) per core
MT = RPC // 128          # 4 m-tiles of 128 query rows
DC = D // 128            # 2 contraction chunks of 128
KBLK = B // 512          # 8 key blocks of 512 keys
T_POS = 0.05
T_NEG = 0.1

F32 = mybir.dt.float32
BF16 = mybir.dt.bfloat16
ALU = mybir.AluOpType
ACTF = mybir.ActivationFunctionType


class _Bacc(bacc.Bacc):
    """Bacc that restricts Ln/Exp to the combined natural_log_exp table set so
    interleaved Ln/Exp emit a single ACT table load instead of thrashing."""

    def insert_act_table_loads(self):
        import bass_rust as _bass_rust
        from concourse.hw_specs import get_activation_tables

        has_activation = any(
            isinstance(i, mybir.InstActivation)
            for b in self.main_func.blocks
            for i in b.instructions
        )
        if not has_activation:
            return
        items = list(get_activation_tables(self.m.arch).items())
        lnexp = {ACTF.Ln, ACTF.Exp}
        tables = [
            (k, v if k == "natural_log_exp_and_others" else (v - lnexp))
            for k, v in items
        ]
        _bass_rust.insert_act_table_loads(self, tables)


def _emit_body(nc, tc, pools, rep, q_d, p_d, denom_d, pos_d, ident):
    const, sb, work, small, dram = pools

    # ---- Phase A: local query-shard preprocessing ----
    q_sb = sb.tile([128, MT, D], F32, tag="q_sb")
    nc.sync.dma_start(
        out=q_sb[:], in_=q_d.ap().rearrange("(t p) d -> p t d", p=128)
    )
    p_sb = sb.tile([128, MT, D], F32, tag="p_sb")
    nc.sync.dma_start(
        out=p_sb[:], in_=p_d.ap().rearrange("(t p) d -> p t d", p=128)
    )

    # ssq for q rows (cols 0..MT) and p rows (cols MT..2MT) in one tile
    ssq = small.tile([128, 2 * MT], F32, tag="ssq")
    for t in range(MT):
        scratch = work.tile([128, D], F32, tag="scr")
        nc.vector.scalar_tensor_tensor(
            out=scratch[:], in0=q_sb[:, t, :], scalar=1.0, in1=q_sb[:, t, :],
            op0=ALU.mult, op1=ALU.mult, accum_out=ssq[:, t : t + 1],
        )
    # inv_norm = exp(-0.5 * ln(ssq)); Ln+Exp share one ACT table set
    # (Rsqrt activation is disallowed for accuracy reasons).
    ln_q = small.tile([128, MT], F32, tag="ln_q")
    nc.scalar.activation(out=ln_q[:], in_=ssq[:, 0:MT], func=ACTF.Ln)
    inv_q = small.tile([128, MT], F32, tag="inv_q")
    nc.scalar.activation(out=inv_q[:], in_=ln_q[:], func=ACTF.Exp, scale=-0.5)

    qn_bf = sb.tile([128, MT, D], BF16, tag="qn_bf")
    for t in range(MT):
        nc.vector.tensor_scalar_mul(
            out=qn_bf[:, t, :], in0=q_sb[:, t, :], scalar1=inv_q[:, t : t + 1],
        )

    # Transpose normalized bf16 queries: qt[d, row] (this core's K^T shard)
    qt_sb = sb.tile([128, DC, RPC], BF16, tag="qt_sb")
    with tc.tile_pool(name=f"tpsum{rep}", bufs=2, space="PSUM") as tpsum:
        for t in range(MT):
            for dc in range(DC):
                pt = tpsum.tile([128, 128], BF16, tag="tp")
                nc.tensor.transpose(
                    pt[:], qn_bf[:, t, dc * 128 : (dc + 1) * 128], ident[:]
                )
                nc.vector.tensor_copy(
                    out=qt_sb[:, dc, t * 128 : (t + 1) * 128], in_=pt[:]
                )

    # ---- AllGather normalized transposed shards ----
    cc_in = dram.tile([DC, 128, RPC], BF16, tag="cc_in")
    nc.sync.dma_start(out=cc_in[:].rearrange("c p r -> p c r"), in_=qt_sb[:])
    cc_out = dram.tile(
        [NCORES * DC, 128, RPC], BF16, addr_space="Shared", tag="cc_out"
    )
    nc.gpsimd.collective_compute(
        "AllGather",
        ALU.bypass,
        replica_groups=[list(range(NCORES))],
        ins=[cc_in[:].opt()],
        outs=[cc_out[:].opt()],
    )

    # ---- During AG: positive-pair dot products + diagonal terms ----
    pos_raw = small.tile([128, MT], F32, tag="pos_raw")
    for t in range(MT):
        scratch = work.tile([128, D], F32, tag="scr")
        nc.vector.scalar_tensor_tensor(
            out=scratch[:], in0=p_sb[:, t, :], scalar=1.0, in1=p_sb[:, t, :],
            op0=ALU.mult, op1=ALU.mult, accum_out=ssq[:, MT + t : MT + t + 1],
        )
        scratch2 = work.tile([128, D], F32, tag="scr")
        nc.vector.scalar_tensor_tensor(
            out=scratch2[:], in0=q_sb[:, t, :], scalar=1.0, in1=p_sb[:, t, :],
            op0=ALU.mult, op1=ALU.mult, accum_out=pos_raw[:, t : t + 1],
        )
    # diag term: sim_ii as the bf16 matmul would compute it
    diag_raw = small.tile([128, MT], F32, tag="diag_raw")
    for t in range(MT):
        scratch3 = work.tile([128, D], BF16, tag="scrb")
        nc.vector.scalar_tensor_tensor(
            out=scratch3[:], in0=qn_bf[:, t, :], scalar=1.0, in1=qn_bf[:, t, :],
            op0=ALU.mult, op1=ALU.mult, accum_out=diag_raw[:, t : t + 1],
        )

    # inv_p (cols MT..2MT of ssq) and diag_exp batched on ACT
    ln_p = small.tile([128, MT], F32, tag="ln_p")
    nc.scalar.activation(out=ln_p[:], in_=ssq[:, MT : 2 * MT], func=ACTF.Ln)
    inv_p = small.tile([128, MT], F32, tag="inv_p")
    nc.scalar.activation(out=inv_p[:], in_=ln_p[:], func=ACTF.Exp, scale=-0.5)
    diag_exp = small.tile([128, MT], F32, tag="diag_exp")
    nc.scalar.activation(
        out=diag_exp[:], in_=diag_raw[:], func=ACTF.Exp, scale=1.0 / T_NEG
    )

    pos_sb = small.tile([128, MT], F32, tag="pos_sb")
    nc.vector.tensor_mul(out=pos_sb[:], in0=pos_raw[:], in1=inv_q[:])
    nc.vector.tensor_mul(out=pos_sb[:], in0=pos_sb[:], in1=inv_p[:])

    # ---- Read back gathered K^T ----
    kt_all = sb.tile([128, NCORES * DC, RPC], BF16, tag="kt_all")
    nchunk = 4
    per = NCORES * DC // nchunk
    for ck in range(nchunk):
        nc.sync.dma_start(
            out=kt_all[:, ck * per : (ck + 1) * per, :],
            in_=cc_out[:].rearrange("c p r -> p c r")[
                :, ck * per : (ck + 1) * per, :
            ],
        )

    # ---- Main: S = qn @ K^T blockwise, exp + fused row-sum ----
    rowsums = small.tile([128, MT, 2], F32, tag="rowsums")
    with tc.tile_pool(name=f"mpsum{rep}", bufs=2, space="PSUM") as mpsum:
        for m in range(MT):
            for half in range(2):
                ps = mpsum.tile([128, 2048], F32, tag="mm")
                for dc in range(DC):
                    for rb in range(4):
                        r = half * 4 + rb
                        nc.tensor.matmul(
                            ps[:, rb * 512 : (rb + 1) * 512],
                            lhsT=qt_sb[:, dc, m * 128 : (m + 1) * 128],
                            rhs=kt_all[:, r * DC + dc, :],
                            start=(dc == 0),
                            stop=(dc == DC - 1),
                        )
                nc.scalar.activation(
                    out=ps[:],
                    in_=ps[:],
                    func=ACTF.Exp,
                    scale=1.0 / T_NEG,
                    accum_out=rowsums[:, m, half : half + 1],
                )

    denom_sb = small.tile([128, MT], F32, tag="denom_sb")
    nc.vector.tensor_add(
        out=denom_sb[:], in0=rowsums[:, :, 0], in1=rowsums[:, :, 1]
    )
    nc.vector.tensor_sub(out=denom_sb[:], in0=denom_sb[:], in1=diag_exp[:])

    nc.sync.dma_start(out=denom_d.ap(), in_=denom_sb[:])
    nc.sync.dma_start(out=pos_d.ap(), in_=pos_sb[:])


def _build(reps=1):
    nc = _Bacc(
        "TRN2", target_bir_lowering=False, debug=False, num_devices=NCORES
    )
    q_d = nc.dram_tensor("q", [RPC, D], F32, kind="ExternalInput")
    p_d = nc.dram_tensor("p", [RPC, D], F32, kind="ExternalInput")
    denom_d = nc.dram_tensor("denom", [128, MT], F32, kind="ExternalOutput")
    pos_d = nc.dram_tensor("pos", [128, MT], F32, kind="ExternalOutput")

    with tile.TileContext(nc) as tc:
        with (
            tc.tile_pool(name="const", bufs=1) as const,
            tc.tile_pool(name="sb", bufs=1) as sb,
            tc.tile_pool(name="work", bufs=2) as work,
            tc.tile_pool(name="small", bufs=1) as small,
            tc.tile_pool(name="dram", bufs=1, space="DRAM") as dram,
        ):
            ident = const.tile([128, 128], BF16)
            make_identity(nc, ident)
            pools = (const, sb, work, small, dram)
            for rep in range(reps):
                _emit_body(nc, tc, pools, rep, q_d, p_d, denom_d, pos_d, ident)

    nc.finalize()
    return nc


_NC_CACHE = []


def _get_nc():
    if not _NC_CACHE:
        _NC_CACHE.append(_build())
    return _NC_CACHE[0]


_RUNNER_CACHE = []


def _make_runner():
    """Build a cached jitted SPMD executor (mirrors bass2jax.run_bass_via_pjrt

    multi-core branch, but reusable across calls so repeat invocations skip
    recompilation)."""
    import jax
    from jax.experimental.shard_map import shard_map
    from jax.sharding import Mesh, PartitionSpec
    import concourse.mybir as _mybir
    from concourse import bass2jax

    nc = _get_nc()
    bass2jax.install_neuronx_cc_hook()

    partition_name = (
        nc.partition_id_tensor.name if nc.partition_id_tensor else None
    )
    in_names = []
    out_names = []
    out_avals = []
    zero_shapes = []
    for alloc in nc.m.functions[0].allocations:
        if not isinstance(alloc, _mybir.MemoryLocationSet):
            continue
        name = alloc.memorylocations[0].name
        if alloc.kind == "ExternalInput":
            if name != partition_name:
                in_names.append(name)
        elif alloc.kind == "ExternalOutput":
            out_names.append(name)
            shape = tuple(alloc.tensor_shape)
            dtype = _mybir.dt.np(alloc.dtype)
            out_avals.append(jax.core.ShapedArray(shape, dtype))
            zero_shapes.append((shape, dtype))
    n_params = len(in_names)
    n_outs = len(out_names)
    all_names = in_names + out_names
    if partition_name is not None:
        all_names = all_names + [partition_name]

    def _body(*args):
        operands = list(args)
        if partition_name is not None:
            operands.append(bass2jax.partition_id_tensor())
        outs = bass2jax._bass_exec_p.bind(
            *operands,
            out_avals=tuple(out_avals),
            in_names=tuple(all_names),
            out_names=tuple(out_names),
            lowering_input_output_aliases=(),
            sim_require_finite=True,
            sim_require_nnan=True,
            nc=nc,
        )
        return tuple(outs)

    devices = jax.devices()[:NCORES]
    mesh = Mesh(np.asarray(devices), ("core",))
    in_specs = (PartitionSpec("core"),) * (n_params + n_outs)
    out_specs = (PartitionSpec("core"),) * n_outs
    donate = tuple(range(n_params, n_params + n_outs))
    sharded = jax.jit(
        shard_map(
            _body, mesh=mesh, in_specs=in_specs, out_specs=out_specs,
            check_rep=False,
        ),
        donate_argnums=donate,
        keep_unused=True,
    )

    def run(in_maps):
        concat_in = [
            np.concatenate([np.asarray(in_maps[c][nm]) for c in range(NCORES)], axis=0)
            for nm in in_names
        ]
        concat_zeros = [
            np.zeros((NCORES * s[0], *s[1:]), dt) for s, dt in zero_shapes
        ]
        out_arrs = sharded(*concat_in, *concat_zeros)
        return [
            {
                nm: np.asarray(out_arrs[i]).reshape(NCORES, *out_avals[i].shape)[c]
                for i, nm in enumerate(out_names)
            }
            for c in range(NCORES)
        ]

    return run


def _get_runner():
    if not _RUNNER_CACHE:
        _RUNNER_CACHE.append(_make_runner())
    return _RUNNER_CACHE[0]


def _in_maps(x):
    return [
        {
            "q": x[c * RPC : (c + 1) * RPC],
            "p": x[B + c * RPC : B + (c + 1) * RPC],
        }
        for c in range(NCORES)
    ]


def _reduce_results(results):
    total = np.float64(0.0)
    for r in results:
        denom = r["denom"].astype(np.float64)
        pos = r["pos"].astype(np.float64)
        total += np.sum(np.log(denom) - pos / T_POS)
    return np.float32(total / B)


def _run(representations, **spmd_kwargs):
    x = np.ascontiguousarray(np.asarray(representations, dtype=np.float32))
    assert x.shape == (2 * B, D), x.shape
    nc = _get_nc()
    res = run_bass_kernel_spmd(
        nc, _in_maps(x), core_ids=list(range(NCORES)), **spmd_kwargs
    )
    return _reduce_results(res.results), res


def kernel(representations):
    x = np.ascontiguousarray(np.asarray(representations, dtype=np.float32))
    assert x.shape == (2 * B, D), x.shape
    results = _get_runner()(_in_maps(x))
    return _reduce_results(results)


if __name__ == "__main__":
    rng = np.random.default_rng(0)
    x = rng.standard_normal((2 * B, D), dtype=np.float32)
    print(kernel(x))


# revision 10
# speedup vs baseline: 5051.8363x; 5051.8363x over previous
"""Contrastive (MixAware) loss kernel for Trainium2, 8 NeuronCores.

Strategy (row-parallel with all-gathered normalized embeddings):
  - x = representations [2B, D] with B=4096, D=256. Rows are split:
    queries q = x[:B], positives p = x[B:].
  - Shard query rows across 8 cores (512 rows each). Core c also gets its
    positive rows p_c.
  - Each core: normalizes its OWN 512 query rows (which are everyone's key
    columns), casts to bf16, transposes via the PE, and AllGathers the
    [D, 512] normalized transposed shard -> full K^T [D, B] in bf16.
  - Each core computes its [512, B] block of the cosine-similarity matrix
    as bf16 matmuls into PSUM, applies exp(./T_NEG) on the Scalar engine
    with fused row-sum accumulation (accum_out), subtracts the diagonal
    term exp(sim_ii/T_NEG) (computed locally from the same bf16 values),
    and computes the positive dot products in fp32.
  - Host sums: loss = sum(log(denom) - pos/T_POS) / B.
"""

import numpy as np

import concourse.bass as bass
import concourse.mybir as mybir
import concourse.tile as tile
from concourse import bacc
from concourse.bass_utils import run_bass_kernel_spmd
from concourse.masks import make_identity

B = 4096
D = 256
NCORES = 8
RPC = B // NCORES        # 512 rows (queries) per core
MT = RPC // 128          # 4 m-tiles of 128 query rows
DC = D // 128            # 2 contraction chunks of 128
KBLK = B // 512          # 8 key blocks of 512 keys
T_POS = 0.05
T_NEG = 0.1

F32 = mybir.dt.float32
BF16 = mybir.dt.bfloat16
ALU = mybir.AluOpType
ACTF = mybir.ActivationFunctionType


class _Bacc(bacc.Bacc):
    """Bacc that restricts Ln/Exp to the combined natural_log_exp table set so
    interleaved Ln/Exp emit a single ACT table load instead of thrashing."""

    def insert_act_table_loads(self):
        import bass_rust as _bass_rust
        from concourse.hw_specs import get_activation_tables

        has_activation = any(
            isinstance(i, mybir.InstActivation)
            for b in self.main_func.blocks
            for i in b.instructions
        )
        if not has_activation:
            return
        items = list(get_activation_tables(self.m.arch).items())
        lnexp = {ACTF.Ln, ACTF.Exp}
        tables = [
            (k, v if k == "natural_log_exp_and_others" else (v - lnexp))
            for k, v in items
        ]
        _bass_rust.insert_act_table_loads(self, tables)


def _emit_body(nc, tc, pools, rep, q_d, p_d, denom_d, pos_d, ident):
    const, sb, work, small, dram = pools

    # ---- Phase A: local query-shard preprocessing ----
    q_sb = sb.tile([128, MT, D], F32, tag="q_sb")
    for qh in range(2):
        hs = slice(qh * (MT // 2), (qh + 1) * (MT // 2))
        nc.sync.dma_start(
            out=q_sb[:, hs, :],
            in_=q_d.ap().rearrange("(t p) d -> p t d", p=128)[:, hs, :],
        )
    p_sb = sb.tile([128, MT, D], F32, tag="p_sb")
    nc.sync.dma_start(
        out=p_sb[:], in_=p_d.ap().rearrange("(t p) d -> p t d", p=128)
    )

    # ssq for q rows (cols 0..MT) and p rows (cols MT..2MT) in one tile
    ssq = small.tile([128, 2 * MT], F32, tag="ssq")
    for t in range(MT):
        scratch = work.tile([128, D], F32, tag="scr")
        nc.vector.scalar_tensor_tensor(
            out=scratch[:], in0=q_sb[:, t, :], scalar=1.0, in1=q_sb[:, t, :],
            op0=ALU.mult, op1=ALU.mult, accum_out=ssq[:, t : t + 1],
        )
    # inv_norm = exp(-0.5 * ln(ssq)); Ln+Exp share one ACT table set
    # (Rsqrt activation is disallowed for accuracy reasons).
    ln_q = small.tile([128, MT], F32, tag="ln_q")
    nc.scalar.activation(out=ln_q[:], in_=ssq[:, 0:MT], func=ACTF.Ln)
    inv_q = small.tile([128, MT], F32, tag="inv_q")
    nc.scalar.activation(out=inv_q[:], in_=ln_q[:], func=ACTF.Exp, scale=-0.5)

    qn_bf = sb.tile([128, MT, D], BF16, tag="qn_bf")
    for t in range(MT):
        nc.vector.tensor_scalar_mul(
            out=qn_bf[:, t, :], in0=q_sb[:, t, :], scalar1=inv_q[:, t : t + 1],
        )

    # Transpose normalized bf16 queries: qt[d, row] (this core's K^T shard)
    qt_sb = sb.tile([128, DC, RPC], BF16, tag="qt_sb")
    with tc.tile_pool(name=f"tpsum{rep}", bufs=2, space="PSUM") as tpsum:
        for t in range(MT):
            for dc in range(DC):
                pt = tpsum.tile([128, 128], BF16, tag="tp")
                nc.tensor.transpose(
                    pt[:], qn_bf[:, t, dc * 128 : (dc + 1) * 128], ident[:]
                )
                nc.vector.tensor_copy(
                    out=qt_sb[:, dc, t * 128 : (t + 1) * 128], in_=pt[:]
                )

    # ---- AllGather normalized transposed shards ----
    cc_in = dram.tile([DC, 128, RPC], BF16, tag="cc_in")
    for dcx in range(DC):
        nc.sync.dma_start(
            out=cc_in[:].rearrange("c p r -> p c r")[:, dcx : dcx + 1, :],
            in_=qt_sb[:, dcx : dcx + 1, :],
        )
    cc_out = dram.tile(
        [NCORES * DC, 128, RPC], BF16, addr_space="Shared", tag="cc_out"
    )
    nc.gpsimd.collective_compute(
        "AllGather",
        ALU.bypass,
        replica_groups=[list(range(NCORES))],
        ins=[cc_in[:].opt()],
        outs=[cc_out[:].opt()],
    )

    # ---- During AG: positive-pair dot products + diagonal terms ----
    pos_raw = small.tile([128, MT], F32, tag="pos_raw")
    for t in range(MT):
        scratch = work.tile([128, D], F32, tag="scr")
        nc.vector.scalar_tensor_tensor(
            out=scratch[:], in0=p_sb[:, t, :], scalar=1.0, in1=p_sb[:, t, :],
            op0=ALU.mult, op1=ALU.mult, accum_out=ssq[:, MT + t : MT + t + 1],
        )
        scratch2 = work.tile([128, D], F32, tag="scr")
        nc.vector.scalar_tensor_tensor(
            out=scratch2[:], in0=q_sb[:, t, :], scalar=1.0, in1=p_sb[:, t, :],
            op0=ALU.mult, op1=ALU.mult, accum_out=pos_raw[:, t : t + 1],
        )
    # diag term: sim_ii as the bf16 matmul would compute it
    diag_raw = small.tile([128, MT], F32, tag="diag_raw")
    for t in range(MT):
        scratch3 = work.tile([128, D], BF16, tag="scrb")
        nc.vector.scalar_tensor_tensor(
            out=scratch3[:], in0=qn_bf[:, t, :], scalar=1.0, in1=qn_bf[:, t, :],
            op0=ALU.mult, op1=ALU.mult, accum_out=diag_raw[:, t : t + 1],
        )

    # inv_p (cols MT..2MT of ssq) and diag_exp batched on ACT
    ln_p = small.tile([128, MT], F32, tag="ln_p")
    nc.scalar.activation(out=ln_p[:], in_=ssq[:, MT : 2 * MT], func=ACTF.Ln)
    inv_p = small.tile([128, MT], F32, tag="inv_p")
    nc.scalar.activation(out=inv_p[:], in_=ln_p[:], func=ACTF.Exp, scale=-0.5)
    diag_exp = small.tile([128, MT], F32, tag="diag_exp")
    nc.scalar.activation(
        out=diag_exp[:], in_=diag_raw[:], func=ACTF.Exp, scale=1.0 / T_NEG
    )

    pos_sb = small.tile([128, MT], F32, tag="pos_sb")
    nc.vector.tensor_mul(out=pos_sb[:], in0=pos_raw[:], in1=inv_q[:])
    nc.vector.tensor_mul(out=pos_sb[:], in0=pos_sb[:], in1=inv_p[:])

    # ---- Read back gathered K^T ----
    kt_all = sb.tile([128, NCORES * DC, RPC], BF16, tag="kt_all")
    nchunk = 4
    per = NCORES * DC // nchunk
    for ck in range(nchunk):
        nc.sync.dma_start(
            out=kt_all[:, ck * per : (ck + 1) * per, :],
            in_=cc_out[:].rearrange("c p r -> p c r")[
                :, ck * per : (ck + 1) * per, :
            ],
        )

    # ---- Main: S = qn @ K^T blockwise, exp + fused row-sum ----
    rowsums = small.tile([128, MT, 2], F32, tag="rowsums")
    with tc.tile_pool(name=f"mpsum{rep}", bufs=2, space="PSUM") as mpsum:
        for m in range(MT):
            for half in range(2):
                ps = mpsum.tile([128, 2048], F32, tag="mm")
                for dc in range(DC):
                    for rb in range(4):
                        r = half * 4 + rb
                        nc.tensor.matmul(
                            ps[:, rb * 512 : (rb + 1) * 512],
                            lhsT=qt_sb[:, dc, m * 128 : (m + 1) * 128],
                            rhs=kt_all[:, r * DC + dc, :],
                            start=(dc == 0),
                            stop=(dc == DC - 1),
                        )
                nc.scalar.activation(
                    out=ps[:],
                    in_=ps[:],
                    func=ACTF.Exp,
                    scale=1.0 / T_NEG,
                    accum_out=rowsums[:, m, half : half + 1],
                )

    denom_sb = small.tile([128, MT], F32, tag="denom_sb")
    nc.vector.tensor_add(
        out=denom_sb[:], in0=rowsums[:, :, 0], in1=rowsums[:, :, 1]
    )
    nc.vector.tensor_sub(out=denom_sb[:], in0=denom_sb[:], in1=diag_exp[:])

    nc.sync.dma_start(out=denom_d.ap(), in_=denom_sb[:])
    nc.sync.dma_start(out=pos_d.ap(), in_=pos_sb[:])


def _build(reps=1):
    nc = _Bacc(
        "TRN2", target_bir_lowering=False, debug=False, num_devices=NCORES
    )
    q_d = nc.dram_tensor("q", [RPC, D], F32, kind="ExternalInput")
    p_d = nc.dram_tensor("p", [RPC, D], F32, kind="ExternalInput")
    denom_d = nc.dram_tensor("denom", [128, MT], F32, kind="ExternalOutput")
    pos_d = nc.dram_tensor("pos", [128, MT], F32, kind="ExternalOutput")

    with tile.TileContext(nc) as tc:
        with (
            tc.tile_pool(name="const", bufs=1) as const,
            tc.tile_pool(name="sb", bufs=1) as sb,
            tc.tile_pool(name="work", bufs=2) as work,
            tc.tile_pool(name="small", bufs=1) as small,
            tc.tile_pool(name="dram", bufs=1, space="DRAM") as dram,
        ):
            ident = const.tile([128, 128], BF16)
            make_identity(nc, ident)
            pools = (const, sb, work, small, dram)
            for rep in range(reps):
                _emit_body(nc, tc, pools, rep, q_d, p_d, denom_d, pos_d, ident)

    nc.finalize()
    return nc


_NC_CACHE = []


def _get_nc():
    if not _NC_CACHE:
        _NC_CACHE.append(_build())
    return _NC_CACHE[0]


_RUNNER_CACHE = []


def _make_runner():
    """Build a cached jitted SPMD executor (mirrors bass2jax.run_bass_via_pjrt

    multi-core branch, but reusable across calls so repeat invocations skip
    recompilation)."""
    import jax
    from jax.experimental.shard_map import shard_map
    from jax.sharding import Mesh, PartitionSpec
    import concourse.mybir as _mybir
    from concourse import bass2jax

    nc = _get_nc()
    bass2jax.install_neuronx_cc_hook()

    partition_name = (
        nc.partition_id_tensor.name if nc.partition_id_tensor else None
    )
    in_names = []
    out_names = []
    out_avals = []
    zero_shapes = []
    for alloc in nc.m.functions[0].allocations:
        if not isinstance(alloc, _mybir.MemoryLocationSet):
            continue
        name = alloc.memorylocations[0].name
        if alloc.kind == "ExternalInput":
            if name != partition_name:
                in_names.append(name)
        elif alloc.kind == "ExternalOutput":
            out_names.append(name)
            shape = tuple(alloc.tensor_shape)
            dtype = _mybir.dt.np(alloc.dtype)
            out_avals.append(jax.core.ShapedArray(shape, dtype))
            zero_shapes.append((shape, dtype))
    n_params = len(in_names)
    n_outs = len(out_names)
    all_names = in_names + out_names
    if partition_name is not None:
        all_names = all_names + [partition_name]

    def _body(*args):
        operands = list(args)
        if partition_name is not None:
            operands.append(bass2jax.partition_id_tensor())
        outs = bass2jax._bass_exec_p.bind(
            *operands,
            out_avals=tuple(out_avals),
            in_names=tuple(all_names),
            out_names=tuple(out_names),
            lowering_input_output_aliases=(),
            sim_require_finite=True,
            sim_require_nnan=True,
            nc=nc,
        )
        return tuple(outs)

    devices = jax.devices()[:NCORES]
    mesh = Mesh(np.asarray(devices), ("core",))
    in_specs = (PartitionSpec("core"),) * (n_params + n_outs)
    out_specs = (PartitionSpec("core"),) * n_outs
    donate = tuple(range(n_params, n_params + n_outs))
    sharded = jax.jit(
        shard_map(
            _body, mesh=mesh, in_specs=in_specs, out_specs=out_specs,
            check_rep=False,
        ),
        donate_argnums=donate,
        keep_unused=True,
    )

    def run(in_maps):
        concat_in = [
            np.concatenate([np.asarray(in_maps[c][nm]) for c in range(NCORES)], axis=0)
            for nm in in_names
        ]
        concat_zeros = [
            np.zeros((NCORES * s[0], *s[1:]), dt) for s, dt in zero_shapes
        ]
        out_arrs = sharded(*concat_in, *concat_zeros)
        return [
            {
                nm: np.asarray(out_arrs[i]).reshape(NCORES, *out_avals[i].shape)[c]
                for i, nm in enumerate(out_names)
            }
            for c in range(NCORES)
        ]

    return run


def _get_runner():
    if not _RUNNER_CACHE:
        _RUNNER_CACHE.append(_make_runner())
    return _RUNNER_CACHE[0]


def _in_maps(x):
    return [
        {
            "q": x[c * RPC : (c + 1) * RPC],
            "p": x[B + c * RPC : B + (c + 1) * RPC],
        }
        for c in range(NCORES)
    ]


def _reduce_results(results):
    total = np.float64(0.0)
    for r in results:
        denom = r["denom"].astype(np.float64)
        pos = r["pos"].astype(np.float64)
        total += np.sum(np.log(denom) - pos / T_POS)
    return np.float32(total / B)


def _run(representations, **spmd_kwargs):
    x = np.ascontiguousarray(np.asarray(representations, dtype=np.float32))
    assert x.shape == (2 * B, D), x.shape
    nc = _get_nc()
    res = run_bass_kernel_spmd(
        nc, _in_maps(x), core_ids=list(range(NCORES)), **spmd_kwargs
    )
    return _reduce_results(res.results), res


def kernel(representations):
    x = np.ascontiguousarray(np.asarray(representations, dtype=np.float32))
    assert x.shape == (2 * B, D), x.shape
    results = _get_runner()(_in_maps(x))
    return _reduce_results(results)


if __name__ == "__main__":
    rng = np.random.default_rng(0)
    x = rng.standard_normal((2 * B, D), dtype=np.float32)
    print(kernel(x))
